# revision 1
# baseline (speedup 1.0000x reference)
"""Contrastive-loss kernel for Trainium2, 8 NeuronCores.

Math
----
reference:
    yn  = ys / clip(||ys||, 1e-6)         (row-normalize)
    cos = yn @ yn.T                        [B, B]
    pair_loss = same ? relu(2 - cos)^2 : cos^2
    loss = sum(strict_lower(pair_loss)) / (B*(B-1)/2)

Because margin M = 2 and |cos| <= 1, relu(2 - cos) == 2 - cos always, so
    pair_loss = cos^2 + 4 * same * (1 - cos)
and since cos / same are symmetric with cos_ii == 1, same_ii == 1:
    sum_{i>j} pair_loss = (F1 - B) / 2 + 2 * F2
where over the FULL matrix
    F1 = sum_ij cos_ij^2
    F2 = sum_ij same_ij * (1 - cos_ij) = sum_ij same_ij - sum_ij same_ij*cos_ij

No triangle masking and no relu are needed: each core computes its
512-row block of the full Gram matrix and three per-partition sums
(sum cos^2, sum same, sum same*cos); the host combines 8x[128] partials.

Device plan (SPMD, identical program on 8 cores; only input data differs):
 1. Each core loads its own 512 rows (f32), computes row norms
    (ACT Square+accum), normalizes, casts to bf16, transposes its
    [512, 2048] shard to K-major [2048, 512] via PE transpose.
 2. AllGather of the bf16 transposed shards -> full ynT [C*2048, 512-blocks].
 3. bf16 Gram matmul: lhsT = own ynT columns, rhs = gathered ynT
    (SBUF-resident), accumulated over K in PSUM (f32).
 4. Epilogue per [128, 512] tile: ACT Square+accum (sum cos^2),
    DVE is_equal+accum (sum same), DVE tensor_tensor_reduce (sum same*cos).
 5. DMA out [128, 4] f32 partials; host reduces.
"""

import os
import sys

for _p in ("/opt/trn_rl_repo", "/root/.axon_site/_ro/trn_rl_repo"):
    if _p not in sys.path and os.path.isdir(_p):
        sys.path.append(_p)

import numpy as np

import concourse.bass as bass
import concourse.mybir as mybir
import concourse.tile as tile
from concourse import masks
from concourse.bass import ds, ts  # noqa: F401

F32 = mybir.dt.float32
BF16 = mybir.dt.bfloat16
AF = mybir.ActivationFunctionType
ALU = mybir.AluOpType

P = 128  # partitions


def _split_multi_waits(nc):
    """Split instructions carrying >1 semaphore wait.

    The walrus in this environment rejects compute instructions with more
    than one sync-wait command ("Too many sync wait commands"). Move the
    extra waits onto standalone EventSemaphore instructions inserted just
    before, on the same engine — semantically identical (the engine's
    sequencer blocks on each in order).
    """
    n_split = 0
    for fn in nc.m.functions:
        for bb in fn.blocks:
            new_insts = []
            for ins in bb.instructions:
                si = ins.sync_info
                if (
                    si is not None
                    and len(si.on_wait) > 1
                    and not isinstance(ins, mybir.InstEventSemaphore)
                ):
                    extra = list(si.on_wait[1:])
                    ins.sync_info = mybir.SyncInfo(
                        on_wait=[si.on_wait[0]], on_update=list(si.on_update)
                    )
                    for w in extra:
                        n_split += 1
                        ev = mybir.InstEventSemaphore(
                            name=f"antsplitwait_{n_split}_{ins.name}",
                            engine=ins.engine,
                            ins=[],
                            outs=[],
                            sync_info=mybir.SyncInfo(on_wait=[w], on_update=[]),
                            bass_nofuse=True,
                        )
                        new_insts.append(ev)
                new_insts.append(ins)
            bb.instructions = new_insts
    return n_split


def build_gram_loss(B=4096, D=2048, C=8, NT=512, S=4):
    """Build the SPMD bass program (one nc, run on C cores).

    B: total rows; D: features; C: cores; NT: N tile of the Gram matmul.
    S: AllGather split factor — the shard is gathered in S column-chunks so
    the Gram matmul can start after the first chunk and overlap the rest.
    Gathered chunk s holds global rows {r*Bs + s*W .. +W} for r in 0..C-1,
    contiguous in SBUF — labels must be host-permuted to match (see
    make_in_maps / column_perm).
    """
    assert B % (C * P) == 0 and D % P == 0 and B % NT == 0
    Bs = B // C          # rows per core
    RT = Bs // P         # 128-row tiles per core
    KC = D // P          # K chunks
    NJ = B // NT         # N tiles over all columns
    assert Bs % S == 0
    W = Bs // S          # chunk width per core
    assert (C * W) % NT == 0 or NT % (C * W) == 0

    nc = bass.Bass(num_devices=C)

    ys_mine = nc.dram_tensor("ys_mine", [Bs, D], F32, kind="ExternalInput")
    labels_all = nc.dram_tensor("labels_all", [1, B], F32, kind="ExternalInput")
    labels_mine = nc.dram_tensor("labels_mine", [RT, P], F32, kind="ExternalInput")
    out_parts = nc.dram_tensor("out_parts", [P, 4], F32, kind="ExternalOutput")

    # Shared scratchpad output is the fast path but only supported for >4 cores
    cc_space = "Shared" if C > 4 else "Local"
    cc_ins = [nc.dram_tensor(f"cc_in{s}", [D, W], BF16) for s in range(S)]
    cc_outs = [
        nc.dram_tensor(f"cc_out{s}", [C * D, W], BF16, addr_space=cc_space)
        for s in range(S)
    ]

    with tile.TileContext(nc) as tc:
        with (
            tc.tile_pool(name="const", bufs=1) as const_pool,
            tc.tile_pool(name="big", bufs=1) as big_pool,
            tc.tile_pool(name="ysin", bufs=2) as ys_pool,
            tc.tile_pool(name="yn", bufs=2) as yn_pool,
            tc.tile_pool(name="sqscr", bufs=1) as sq_scratch_pool,
            tc.tile_pool(name="small", bufs=4) as small_pool,
            tc.tile_pool(name="acc", bufs=1) as acc_pool,
            tc.tile_pool(name="ep", bufs=3) as ep_pool,
            tc.tile_pool(name="red", bufs=6) as red_pool,
            tc.tile_pool(name="pt", bufs=2, space="PSUM") as pt_psum,
            tc.tile_pool(name="mm", bufs=5, space="PSUM") as mm_psum,
            tc.tile_pool(name="lab", bufs=1, space="PSUM") as lab_psum,
        ):
            # ---------------- constants / label prep ----------------
            identity = const_pool.tile([P, P], BF16)
            masks.make_identity(nc, identity[:])

            ones_1xP = const_pool.tile([1, P], BF16)
            nc.gpsimd.memset(ones_1xP[:], 1.0)

            eps_tile = const_pool.tile([P, 1], F32)
            nc.gpsimd.memset(eps_tile[:], 1e-6)

            # own labels, per-partition: [P, RT] f32 (tensor_scalar is_equal
            # requires an f32 scalar operand; values 0..9 are exact)
            l_mine = const_pool.tile([P, RT], F32)
            nc.gpsimd.dma_start(
                out=l_mine[:], in_=labels_mine[:, :].rearrange("t p -> p t")
            )

            # all labels on one partition, bf16
            lab_row = const_pool.tile([1, B], BF16)
            nc.gpsimd.dma_start(out=lab_row[:], in_=labels_all[:, :])

            # broadcast labels across partitions: L_col[p, j] = label[j]
            L_col = big_pool.tile([P, B], BF16)
            for jb in range(B // NT):
                ps_lab = lab_psum.tile([P, NT], F32)
                nc.tensor.matmul(
                    ps_lab[:],
                    lhsT=ones_1xP[:],
                    rhs=lab_row[:, ts(jb, NT)],
                    start=True,
                    stop=True,
                )
                nc.scalar.copy(L_col[:, ts(jb, NT)], ps_lab[:])

            # accumulators
            acc_sq = acc_pool.tile([P, 1], F32)
            acc_eq = acc_pool.tile([P, 1], F32)
            acc_eqc = acc_pool.tile([P, 1], F32)
            nc.vector.memset(acc_sq[:], 0.0)
            nc.vector.memset(acc_eq[:], 0.0)
            nc.vector.memset(acc_eqc[:], 0.0)

            # ---------------- phase A: normalize + transpose own shard ----
            ynT_mine = big_pool.tile([P, KC, Bs], BF16)

            for t in range(RT):
                ys_t = ys_pool.tile([P, D], F32)
                # alternate HWDGE rings (SP / ACT) so big DMAs don't
                # serialize on one FIFO
                dmae = nc.sync if t % 2 == 0 else nc.scalar
                dmae.dma_start(out=ys_t[:], in_=ys_mine[ts(t, P), :])

                yn_t = yn_pool.tile([P, D], BF16)
                ssq = small_pool.tile([P, 1], F32)
                # dedicated scratch for the squares: aliasing yn_t here adds a
                # cross-engine WAW dep that overflows the ISA sync-wait slots
                sq_scratch = sq_scratch_pool.tile([P, D], BF16)
                nc.scalar.activation(
                    sq_scratch[:], ys_t[:], AF.Square, accum_out=ssq[:]
                )
                norm_t = small_pool.tile([P, 1], F32)
                nc.scalar.sqrt(norm_t[:], ssq[:])
                normc = small_pool.tile([P, 1], F32)
                # max against a memset tile: a float immediate here lowers to
                # a const-AP read whose extra dep overflows ISA sync-wait slots
                nc.vector.tensor_tensor(
                    normc[:], norm_t[:], eps_tile[:], ALU.max
                )
                r_t = small_pool.tile([P, 1], F32)
                nc.vector.reciprocal(r_t[:], normc[:])
                nc.vector.tensor_scalar_mul(yn_t[:], ys_t[:], r_t[:])

                for kc in range(KC):
                    pt = pt_psum.tile([P, P], BF16)
                    nc.tensor.transpose(pt[:], yn_t[:, ts(kc, P)], identity[:])
                    ev = nc.scalar if kc % 2 == 0 else nc.vector
                    if ev is nc.scalar:
                        nc.scalar.copy(ynT_mine[:, kc, ts(t, P)], pt[:])
                    else:
                        nc.vector.tensor_copy(ynT_mine[:, kc, ts(t, P)], pt[:])

            # ship shard chunks to DRAM and gather them one chunk at a time;
            # the matmul starts once chunk 0 is in SBUF
            rhs_all = big_pool.tile([P, KC, B], BF16)
            for s in range(S):
                nc.sync.dma_start(
                    out=cc_ins[s][:, :].rearrange("(kc p) c -> p kc c", p=P),
                    in_=ynT_mine[:, :, ts(s, W)],
                )
            for s in range(S):
                nc.gpsimd.collective_compute(
                    "AllGather",
                    ALU.bypass,
                    replica_groups=[list(range(C))],
                    ins=[cc_ins[s][:, :]],
                    outs=[cc_outs[s][:, :]],
                )
                for r in range(C):
                    dmae = nc.sync if r % 2 == 0 else nc.scalar
                    dmae.dma_start(
                        out=rhs_all[:, :, s * C * W + r * W : s * C * W + (r + 1) * W],
                        in_=cc_outs[s][r * D : (r + 1) * D, :].rearrange(
                            "(kc p) c -> p kc c", p=P
                        ),
                    )

            # ---------------- phase C: Gram + epilogue ----------------
            # j outer: all row-tiles of an AllGather chunk's columns run
            # before any tile that needs a later chunk — the PE never
            # stalls on a not-yet-gathered chunk while ready work exists
            for j in range(NJ):
                for i in range(RT):
                    ps = mm_psum.tile([P, NT], F32)
                    for kc in range(KC):
                        nc.tensor.matmul(
                            ps[:],
                            lhsT=ynT_mine[:, kc, ts(i, P)],
                            rhs=rhs_all[:, kc, ts(j, NT)],
                            start=(kc == 0),
                            stop=(kc == KC - 1),
                        )

                    # S1 += sum cos^2
                    sq_scr = ep_pool.tile([P, NT], BF16, tag="sq")
                    sq_red = red_pool.tile([P, 1], F32, tag="sqr")
                    nc.scalar.activation(
                        sq_scr[:], ps[:], AF.Square, accum_out=sq_red[:]
                    )
                    nc.vector.tensor_tensor(
                        acc_sq[:], acc_sq[:], sq_red[:], ALU.add
                    )

                    # SB += sum same
                    eq_t = ep_pool.tile([P, NT], BF16, tag="eq")
                    eq_red = red_pool.tile([P, 1], F32, tag="eqr")
                    nc.vector.tensor_scalar(
                        eq_t[:],
                        L_col[:, ts(j, NT)],
                        l_mine[:, i : i + 1],
                        None,
                        ALU.is_equal,
                        op1=ALU.add,
                        accum_out=eq_red[:],
                    )
                    nc.vector.tensor_tensor(
                        acc_eq[:], acc_eq[:], eq_red[:], ALU.add
                    )

                    # SC += sum same * cos
                    # (tensor_tensor_reduce lowers to a raw-ISA op this
                    # walrus rejects — use mult + tensor_reduce instead)
                    eqc_scr = ep_pool.tile([P, NT], F32, tag="eqc")
                    eqc_red = red_pool.tile([P, 1], F32, tag="eqcr")
                    nc.vector.tensor_tensor(
                        eqc_scr[:], eq_t[:], ps[:], ALU.mult
                    )
                    nc.vector.tensor_reduce(
                        eqc_red[:], eqc_scr[:], mybir.AxisListType.X, ALU.add
                    )
                    nc.vector.tensor_tensor(
                        acc_eqc[:], acc_eqc[:], eqc_red[:], ALU.add
                    )

            # ---------------- phase D: write partials ----------------
            out_sb = const_pool.tile([P, 4], F32)
            nc.vector.memset(out_sb[:], 0.0)
            nc.scalar.copy(out_sb[:, 0:1], acc_sq[:])
            nc.scalar.copy(out_sb[:, 1:2], acc_eq[:])
            nc.scalar.copy(out_sb[:, 2:3], acc_eqc[:])
            nc.sync.dma_start(out=out_parts[:, :], in_=out_sb[:])

    _split_multi_waits(nc)
    return nc


def column_perm(B, C, S):
    """Global row index held at each SBUF rhs column (see build_gram_loss)."""
    Bs = B // C
    W = Bs // S
    idx = np.arange(B)
    s, r, c = idx // (C * W), (idx // W) % C, idx % W
    return r * Bs + s * W + c


def make_in_maps(ys, labels, B, D, C, S=4):
    """Shard host inputs into per-core input maps."""
    ys = np.ascontiguousarray(ys, dtype=np.float32)
    lab_f = labels.astype(np.float32)
    Bs = B // C
    RT = Bs // P
    lab_all = lab_f[column_perm(B, C, S)].reshape(1, B)
    in_maps = []
    for k in range(C):
        in_maps.append(
            {
                "ys_mine": ys[k * Bs : (k + 1) * Bs],
                "labels_all": lab_all,
                "labels_mine": lab_f[k * Bs : (k + 1) * Bs].reshape(RT, P),
            }
        )
    return in_maps


def combine_parts(parts_list, B):
    """parts_list: per-core [128, 4] f32 partials -> scalar loss."""
    s1 = 0.0
    sb = 0.0
    sc = 0.0
    for p in parts_list:
        p = np.asarray(p, dtype=np.float64)
        s1 += p[:, 0].sum()
        sb += p[:, 1].sum()
        sc += p[:, 2].sum()
    f2 = sb - sc
    total = (s1 - B) / 2.0 + 2.0 * f2
    n_pair = B * (B - 1) // 2
    return np.float32(total / n_pair)


_CACHED = {}


def kernel(ys: np.ndarray, labels: np.ndarray) -> np.ndarray:
    B, D = ys.shape
    C = 8
    S = 4
    key = (B, D, C, S)
    if key not in _CACHED:
        _CACHED[key] = build_gram_loss(B=B, D=D, C=C, S=S)
    nc = _CACHED[key]

    from concourse.bass_utils import run_bass_kernel_spmd

    in_maps = make_in_maps(np.asarray(ys), np.asarray(labels), B, D, C, S=S)
    res = run_bass_kernel_spmd(nc, in_maps, core_ids=list(range(C)))
    parts = [res.results[i]["out_parts"] for i in range(C)]
    return combine_parts(parts, B)


if __name__ == "__main__":
    # quick smoke: build only
    nc = build_gram_loss()
    print("built ok:", len(nc.m.functions[0].blocks), "blocks")



# revision 7
# speedup vs baseline: 4.7019x; 4.7019x over previous
"""Contrastive-loss kernel for Trainium2, 8 NeuronCores.

Math
----
reference:
    yn  = ys / clip(||ys||, 1e-6)         (row-normalize)
    cos = yn @ yn.T                        [B, B]
    pair_loss = same ? relu(2 - cos)^2 : cos^2
    loss = sum(strict_lower(pair_loss)) / (B*(B-1)/2)

Because margin M = 2 and |cos| <= 1, relu(2 - cos) == 2 - cos always, so
    pair_loss = cos^2 + 4 * same * (1 - cos)
and with cos/same symmetric, cos_ii == 1, same_ii == 1:
    sum_{i>j} pair_loss = (F1 - B) / 2 + 2 * (SB - SC)
where over the FULL matrix
    F1 = sum_ij cos_ij^2  =  ||yn yn^T||_F^2  =  ||yn^T yn||_F^2
    SB = sum_ij same_ij   =  sum_c n_c^2                  (host, from labels)
    SC = sum_ij same_ij cos_ij = sum_c ||u_c||^2,  u_c = sum_{i: l_i=c} yn_i

Key idea: F1 only needs M = yn^T yn, a [D, D] = [2048, 2048] matrix — HALF
the FLOPs of the [B, B] Gram matrix, no PE transposes (yn is naturally
K-major for this contraction), and no per-tile label masking at all.
M = sum over row-shards of yn_m^T yn_m, so each core computes a full M from
only its OWN 512 rows (no gather!), then ONE ReduceScatter sums the partial
Ms and hands each core a disjoint 256-row slice to square-and-sum.
SC needs u_c = class-sums of yn rows: a [10, 2048] one-hot matmul per core
over its own rows; host sums the 8 partials and takes squared norms.

Device plan (SPMD, identical program on 8 cores):
 1. Load own 512 rows (f32), row-normalize (ACT Square+accum, sqrt, max-eps,
    reciprocal, scalar-mul) -> yn bf16 [128, 4, 2048] in SBUF.
 2. u matmul: one-hot lhsT [128, 10] x yn chunks -> PSUM [10, 2048] f32,
    DMA straight to u_out DRAM.
 3. M matmul: for each of 16 d-tiles x 4 n-chunks, accumulate 4 B-chunks in
    a [128, 512] f32 PSUM tile, DMA PSUM -> M DRAM directly (no SBUF copy).
 4. Two ReduceScatters (M rows 0:1024, 1024:2048) pipelined behind the
    matmul: each core receives two [128, 2048] f32 reduced slices.
 5. ACT Square+accum over the slices -> [128, 1] partials -> out_parts.
Host combines: F1 from partials, U = sum of u partials -> SC, SB from
labels, closed form above.
"""

import os
import sys

for _p in ("/opt/trn_rl_repo", "/root/.axon_site/_ro/trn_rl_repo"):
    if _p not in sys.path and os.path.isdir(_p):
        sys.path.append(_p)

import numpy as np

import concourse.bass as bass
import concourse.mybir as mybir
import concourse.tile as tile
from concourse.bass import ds, ts  # noqa: F401

F32 = mybir.dt.float32
BF16 = mybir.dt.bfloat16
AF = mybir.ActivationFunctionType
ALU = mybir.AluOpType

P = 128  # partitions
NCLS = 10  # label classes (randint 0..9)


def _split_multi_waits(nc):
    """Split instructions carrying >1 semaphore wait.

    The walrus in this environment rejects compute instructions with more
    than one sync-wait command ("Too many sync wait commands"). Move the
    extra waits onto standalone EventSemaphore instructions inserted just
    before, on the same engine — semantically identical (the engine's
    sequencer blocks on each in order).
    """
    n_split = 0
    for fn in nc.m.functions:
        for bb in fn.blocks:
            new_insts = []
            for ins in bb.instructions:
                si = ins.sync_info
                if (
                    si is not None
                    and len(si.on_wait) > 1
                    and not isinstance(ins, mybir.InstEventSemaphore)
                ):
                    extra = list(si.on_wait[1:])
                    ins.sync_info = mybir.SyncInfo(
                        on_wait=[si.on_wait[0]], on_update=list(si.on_update)
                    )
                    for w in extra:
                        n_split += 1
                        ev = mybir.InstEventSemaphore(
                            name=f"antsplitwait_{n_split}_{ins.name}",
                            engine=ins.engine,
                            ins=[],
                            outs=[],
                            sync_info=mybir.SyncInfo(on_wait=[w], on_update=[]),
                            bass_nofuse=True,
                        )
                        new_insts.append(ev)
                new_insts.append(ins)
            bb.instructions = new_insts
    return n_split


def build_gram_loss(B=4096, D=2048, C=8, S=2):
    """Build the SPMD bass program (one nc, run on C cores).

    B: total rows; D: features; C: cores; S: number of ReduceScatter chunks
    the [D, D] partial-M matrix is split into (pipelines the collective
    behind the matmul).
    """
    assert B % (C * P) == 0 and D % P == 0
    Bs = B // C          # rows per core
    NCH = Bs // P        # 128-row chunks per core
    DT = D // P          # 128-row d-tiles of M
    NT = 512             # matmul N tile (one PSUM bank of f32)
    NN = D // NT         # n tiles
    assert DT % S == 0
    DTS = DT // S        # d-tiles per RS chunk
    RR = D // C          # M rows per core after ReduceScatter
    assert RR % P == 0 and (DTS * P) % C == 0

    nc = bass.Bass(num_devices=C)

    ys_mine = nc.dram_tensor("ys_mine", [Bs, D], F32, kind="ExternalInput")
    onehot_mine = nc.dram_tensor(
        "onehot_mine", [P, NCH * NCLS], F32, kind="ExternalInput"
    )
    u_out = nc.dram_tensor("u_out", [NCLS, D], F32, kind="ExternalOutput")
    out_parts = nc.dram_tensor("out_parts", [P, 1], F32, kind="ExternalOutput")

    # bf16 collective payload: the NRT reduces in f32 internally; bf16
    # rounding of M adds ~2e-7 relative error to the loss (|M| <= ~2,
    # F1 perturbation ~ 2*sum(M*eps) ~ 1e0 absolute vs 3.4e6 numerator).
    m_chunks = [
        nc.dram_tensor(f"m_part{i}", [DTS * P, D], BF16) for i in range(S)
    ]
    rs_outs = [
        nc.dram_tensor(f"rs_out{i}", [DTS * P // C, D], BF16) for i in range(S)
    ]

    with tile.TileContext(nc) as tc:
        with (
            tc.tile_pool(name="const", bufs=1) as const_pool,
            tc.tile_pool(name="yn", bufs=1) as yn_pool,
            tc.tile_pool(name="ysin", bufs=2) as ys_pool,
            tc.tile_pool(name="sqscr", bufs=1) as sq_scratch_pool,
            tc.tile_pool(name="small", bufs=4) as small_pool,
            tc.tile_pool(name="rsin", bufs=2) as rs_pool,
            tc.tile_pool(name="stg", bufs=6) as stg_pool,
            tc.tile_pool(name="acc", bufs=1) as acc_pool,
            tc.tile_pool(name="mm", bufs=4, space="PSUM") as mm_psum,
        ):
            eps_tile = const_pool.tile([P, 1], F32)
            nc.gpsimd.memset(eps_tile[:], 1e-6)

            # one-hot of own labels, bf16 for the u matmul
            oh_f = const_pool.tile([P, NCH * NCLS], F32)
            nc.gpsimd.dma_start(out=oh_f[:], in_=onehot_mine[:, :])
            oh = const_pool.tile([P, NCH, NCLS], BF16)
            nc.scalar.copy(
                oh[:, :, :].rearrange("p c n -> p (c n)"), oh_f[:]
            )

            # ---------------- phase A: normalize own rows ----------------
            yn_sb = yn_pool.tile([P, NCH, D], BF16)
            for ch in range(NCH):
                ys_t = ys_pool.tile([P, D], F32)
                dmae = nc.sync if ch % 2 == 0 else nc.scalar
                dmae.dma_start(out=ys_t[:], in_=ys_mine[ts(ch, P), :])

                ssq = small_pool.tile([P, 1], F32)
                sq_scratch = sq_scratch_pool.tile([P, D], BF16)
                nc.scalar.activation(
                    sq_scratch[:], ys_t[:], AF.Square, accum_out=ssq[:]
                )
                norm_t = small_pool.tile([P, 1], F32)
                nc.scalar.sqrt(norm_t[:], ssq[:])
                normc = small_pool.tile([P, 1], F32)
                nc.vector.tensor_tensor(normc[:], norm_t[:], eps_tile[:], ALU.max)
                r_t = small_pool.tile([P, 1], F32)
                nc.vector.reciprocal(r_t[:], normc[:])
                nc.vector.tensor_scalar_mul(yn_sb[:, ch, :], ys_t[:], r_t[:])

            # ---------------- phase B: u matmul (class sums) ----------
            for nb in range(NN):
                ps_u = mm_psum.tile([NCLS, NT], F32, tag="u")
                for ch in range(NCH):
                    nc.tensor.matmul(
                        ps_u[:],
                        lhsT=oh[:, ch, :],
                        rhs=yn_sb[:, ch, ts(nb, NT)],
                        start=(ch == 0),
                        stop=(ch == NCH - 1),
                    )
                u_sb = stg_pool.tile([NCLS, NT], F32, tag="ustg")
                if nb % 2 == 0:
                    nc.scalar.copy(u_sb[:], ps_u[:])
                else:
                    nc.vector.tensor_copy(u_sb[:], ps_u[:])
                dmae = nc.sync if nb % 2 == 0 else nc.scalar
                dmae.dma_start(out=u_out[:, ts(nb, NT)], in_=u_sb[:])

            # ---------------- phase C: M = yn^T yn, PSUM -> DRAM ------
            for t in range(DT):
                si = t // DTS
                tr = t % DTS
                for nb in range(NN):
                    ps = mm_psum.tile([P, NT], F32, tag="mm")
                    for ch in range(NCH):
                        nc.tensor.matmul(
                            ps[:],
                            lhsT=yn_sb[:, ch, ts(t, P)],
                            rhs=yn_sb[:, ch, ts(nb, NT)],
                            start=(ch == 0),
                            stop=(ch == NCH - 1),
                        )
                    m_sb = stg_pool.tile([P, NT], BF16, tag="mstg")
                    if nb % 2 == 0:
                        nc.scalar.copy(m_sb[:], ps[:])
                    else:
                        nc.vector.tensor_copy(m_sb[:], ps[:])
                    dmae = nc.sync if nb % 2 == 0 else nc.scalar
                    dmae.dma_start(
                        out=m_chunks[si][ts(tr, P), ts(nb, NT)], in_=m_sb[:]
                    )

            # ---------------- phase D: ReduceScatter chunks -----------
            for si in range(S):
                nc.gpsimd.collective_compute(
                    "ReduceScatter",
                    ALU.add,
                    replica_groups=[list(range(C))],
                    ins=[m_chunks[si][:, :]],
                    outs=[rs_outs[si][:, :]],
                )

            # ---------------- phase E: square reduced slices ----------
            acc_f1 = acc_pool.tile([P, 1], F32)
            nc.vector.memset(acc_f1[:], 0.0)
            RPT = DTS * P // C  # rows per core per RS chunk
            for si in range(S):
                for rt in range(RPT // P) if RPT >= P else [0]:
                    rs_t = rs_pool.tile([P, D], BF16)
                    dmae = nc.sync if si % 2 == 0 else nc.scalar
                    if RPT >= P:
                        dmae.dma_start(
                            out=rs_t[:], in_=rs_outs[si][ts(rt, P), :]
                        )
                        sq_scr = sq_scratch_pool.tile([P, D], BF16)
                        f1_red = small_pool.tile([P, 1], F32)
                        nc.scalar.activation(
                            sq_scr[:], rs_t[:], AF.Square, accum_out=f1_red[:]
                        )
                        nc.vector.tensor_tensor(
                            acc_f1[:], acc_f1[:], f1_red[:], ALU.add
                        )
                    else:
                        # RPT < 128 rows: single partial-height tile
                        rs_p = rs_pool.tile([RPT, D], BF16)
                        dmae.dma_start(out=rs_p[:], in_=rs_outs[si][:, :])
                        sq_scr = sq_scratch_pool.tile([RPT, D], BF16)
                        f1_red = small_pool.tile([RPT, 1], F32)
                        nc.scalar.activation(
                            sq_scr[:], rs_p[:], AF.Square, accum_out=f1_red[:]
                        )
                        nc.vector.tensor_tensor(
                            acc_f1[:RPT], acc_f1[:RPT], f1_red[:], ALU.add
                        )

            nc.sync.dma_start(out=out_parts[:, :], in_=acc_f1[:])

    _split_multi_waits(nc)
    return nc


def make_in_maps(ys, labels, B, D, C, S=2):
    """Shard host inputs into per-core input maps."""
    ys = np.ascontiguousarray(ys, dtype=np.float32)
    labels = np.asarray(labels).astype(np.int64)
    Bs = B // C
    NCH = Bs // P
    in_maps = []
    for k in range(C):
        lab_k = labels[k * Bs : (k + 1) * Bs].reshape(NCH, P)
        # onehot_mine[p, ch*NCLS + c] = (label[ch*128 + p] == c)
        oh = np.zeros((P, NCH * NCLS), dtype=np.float32)
        for ch in range(NCH):
            oh[np.arange(P), ch * NCLS + lab_k[ch]] = 1.0
        in_maps.append(
            {
                "ys_mine": ys[k * Bs : (k + 1) * Bs],
                "onehot_mine": oh,
            }
        )
    return in_maps


def combine_parts(results_list, labels, B):
    """results_list: per-core dicts with out_parts [128,1], u_out [10,D]."""
    f1 = 0.0
    u = None
    for res in results_list:
        f1 += np.asarray(res["out_parts"], dtype=np.float64).sum()
        uk = np.asarray(res["u_out"], dtype=np.float64)
        u = uk if u is None else u + uk
    labels = np.asarray(labels).astype(np.int64)
    counts = np.bincount(labels, minlength=NCLS)
    sb = float((counts.astype(np.float64) ** 2).sum())
    sc = float((u**2).sum())
    total = (f1 - B) / 2.0 + 2.0 * (sb - sc)
    n_pair = B * (B - 1) // 2
    return np.float32(total / n_pair)


_CACHED = {}


def kernel(ys: np.ndarray, labels: np.ndarray) -> np.ndarray:
    B, D = ys.shape
    C = 8
    S = 2
    key = (B, D, C, S)
    if key not in _CACHED:
        _CACHED[key] = build_gram_loss(B=B, D=D, C=C, S=S)
    nc = _CACHED[key]

    from concourse.bass_utils import run_bass_kernel_spmd

    in_maps = make_in_maps(np.asarray(ys), np.asarray(labels), B, D, C, S=S)
    res = run_bass_kernel_spmd(nc, in_maps, core_ids=list(range(C)))
    return combine_parts(
        [res.results[i] for i in range(C)], np.asarray(labels), B
    )


if __name__ == "__main__":
    nc = build_gram_loss()
    print("built ok:", len(nc.m.functions[0].blocks), "blocks")
